# revision 27
# baseline (speedup 1.0000x reference)
"""DotLoss kernel for Trainium2, data-parallel over 8 NeuronCores.

loss = mean_i[ relu(1 + dot(img[I[i]], aud[i]) - dot(img[i], aud[i]))
             + relu(1 + dot(img[i], aud[A[i]]) - dot(img[i], aud[i])) ]

Each core handles N/8 = 4096 rows: local rows stream in via contiguous
HWDGE DMAs (2MB per dma_start, 16KB contiguous per partition), impostor
rows via SWDGE dma_gather (1024 rows per call) from the full (replicated)
embedding tables in device DRAM. Row dots are fused multiply+reduce
(scalar_tensor_tensor) on the vector engine. Each core emits a [128,1]
partial hinge-sum; the host sums partials and divides by N.

Row mapping: chunk k holds rows k*CHUNK + p*SLOTS + c at (partition p,
slot c) — contiguous per partition for big DMA descriptors. dma_gather
position i lands at partition i%128, slot i//128, so the host permutes
each chunk's impostor indices with i = c*128 + p. The summed loss is
permutation-invariant; only the per-row triple alignment matters.
"""

import numpy as np

N, D = 32768, 512
NCORES = 8
SHARD = N // NCORES          # 4096 rows per core
P = 128
# Chunk sizes (rows): big chunks amortize gather descriptor-gen overhead;
# small final chunks shorten the critical tail (last gather drain + the
# DVE work that can only start after it).
CHUNKS = (512,) * 7 + (256, 256)
assert sum(CHUNKS) == SHARD
TSLOTS = SHARD // P          # 32 accumulator columns

_CACHE = {}


def _build_nc():
    import concourse.bacc as bacc
    import concourse.mybir as mybir
    import concourse.tile as tile
    from concourse import library_config
    from contextlib import ExitStack

    fp32 = mybir.dt.float32
    i16 = mybir.dt.int16

    nc = bacc.Bacc("TRN2")
    img_full = nc.dram_tensor("img_full", [N, D], fp32, kind="ExternalInput")
    aud_full = nc.dram_tensor("aud_full", [N, D], fp32, kind="ExternalInput")
    img_loc = nc.dram_tensor("img_loc", [SHARD, D], fp32, kind="ExternalInput")
    aud_loc = nc.dram_tensor("aud_loc", [SHARD, D], fp32, kind="ExternalInput")
    iidx = nc.dram_tensor("iidx", [P, SHARD // 16], i16, kind="ExternalInput")
    aidx = nc.dram_tensor("aidx", [P, SHARD // 16], i16, kind="ExternalInput")
    partial = nc.dram_tensor("partial", [P, 1], fp32, kind="ExternalOutput")

    img_loc_f = img_loc.rearrange("s d -> (s d)")
    aud_loc_f = aud_loc.rearrange("s d -> (s d)")

    mult = mybir.AluOpType.mult
    add = mybir.AluOpType.add
    amax = mybir.AluOpType.max

    with ExitStack() as ctx:
        tc = ctx.enter_context(tile.TileContext(nc))
        lio = ctx.enter_context(tc.tile_pool(name="lio", bufs=4))
        gio = ctx.enter_context(tc.tile_pool(name="gio", bufs=6))
        idxp = ctx.enter_context(tc.tile_pool(name="idxp", bufs=1))
        acc = ctx.enter_context(tc.tile_pool(name="acc", bufs=1))
        scr = ctx.enter_context(tc.tile_pool(name="scr", bufs=6))

        # Load the mlp GPSIMD library first: the Q7 ucode fetch takes ~15us
        # and gates the first dma_gather, so start it as early as possible.
        nc.gpsimd.load_library(library_config.mlp)

        iidx_sb = idxp.tile([P, SHARD // 16], i16, tag="iidx")
        nc.sync.dma_start(out=iidx_sb[:], in_=iidx[:])
        aidx_sb = idxp.tile([P, SHARD // 16], i16, tag="aidx")
        nc.sync.dma_start(out=aidx_sb[:], in_=aidx[:])

        anchor = acc.tile([P, TSLOTS], fp32, tag="anchor")
        iimp = acc.tile([P, TSLOTS], fp32, tag="iimp")
        aimp = acc.tile([P, TSLOTS], fp32, tag="aimp")

        def dot(dst_col, a, b):
            pr = scr.tile([P, D], fp32, tag="pr")
            nc.vector.scalar_tensor_tensor(
                out=pr[:], in0=a, scalar=1.0, in1=b,
                op0=mult, op1=mult, accum_out=dst_col,
            )

        row0 = 0
        col0 = 0
        for k, chunk in enumerate(CHUNKS):
            slots = chunk // P
            ic = chunk // 16
            i0 = row0 // 16
            # Alternate the two SWDGE queues: the SDMA engines round-robin
            # across queues with pending work at packet granularity, so two
            # gather queues get 2/3 of engine time vs the HWDGE local queue
            # and the gather drains keep pace with descriptor generation.
            gi = gio.tile([P, slots, D], fp32, tag="gi")
            nc.gpsimd.dma_gather(
                out_ap=gi[:], in_ap=img_full[:],
                idxs_ap=iidx_sb[:, i0:i0 + ic],
                num_idxs=chunk, num_idxs_reg=chunk, elem_size=D,
            )
            ga = gio.tile([P, slots, D], fp32, tag="ga")
            nc.gpsimd.dma_gather(
                out_ap=ga[:], in_ap=aud_full[:],
                idxs_ap=aidx_sb[:, i0:i0 + ic],
                num_idxs=chunk, num_idxs_reg=chunk, elem_size=D,
            )
            # local chunk: partition p holds rows row0 + p*slots ... +slots,
            # i.e. slots*D contiguous elements starting at (row0 + p*slots)*D
            li = lio.tile([P, slots, D], fp32, tag="li")
            nc.sync.dma_start(
                out=li[:].rearrange("p c d -> p (c d)"),
                in_=img_loc_f[row0 * D:(row0 + chunk) * D].rearrange(
                    "(p e) -> p e", p=P))
            la = lio.tile([P, slots, D], fp32, tag="la")
            nc.sync.dma_start(
                out=la[:].rearrange("p c d -> p (c d)"),
                in_=aud_loc_f[row0 * D:(row0 + chunk) * D].rearrange(
                    "(p e) -> p e", p=P))

            # anchors first: they only need the local chunk, so the DVE has
            # work while this chunk's gathers drain.
            for c in range(slots):
                col = col0 + c
                dot(anchor[:, col:col + 1], li[:, c], la[:, c])
            for c in range(slots):
                col = col0 + c
                dot(iimp[:, col:col + 1], gi[:, c], la[:, c])
            for c in range(slots):
                col = col0 + c
                dot(aimp[:, col:col + 1], li[:, c], ga[:, c])
            row0 += chunk
            col0 += slots

        diff = acc.tile([P, 2 * TSLOTS], fp32, tag="diff")
        nc.vector.tensor_sub(diff[:, 0:TSLOTS], iimp[:], anchor[:])
        nc.vector.tensor_sub(diff[:, TSLOTS:], aimp[:], anchor[:])
        hout = acc.tile([P, 2 * TSLOTS], fp32, tag="hout")
        nc.vector.tensor_scalar(
            out=hout[:], in0=diff[:], scalar1=1.0, scalar2=0.0,
            op0=add, op1=amax,
        )
        psum_t = acc.tile([P, 1], fp32, tag="psum")
        nc.vector.tensor_reduce(
            out=psum_t[:], in_=hout[:], axis=mybir.AxisListType.X, op=add,
        )
        nc.sync.dma_start(out=partial[:], in_=psum_t[:])

    nc.compile()
    return nc


def _get_nc():
    if "nc" not in _CACHE:
        _CACHE["nc"] = _build_nc()
    return _CACHE["nc"]


def _prep_idx(imp_core):
    """Wrap one core's impostor indices into the dma_gather SBUF layout.

    Local row j = row0 + p*slots + c (chunk k starting at row0) is gathered
    by chunk k at position i = c*128 + p. The wrapped tile stores position
    i of chunk k at [i % 16, row0//16 + i // 16], replicated across the 8
    GPSIMD partition groups.
    """
    cols = []
    row0 = 0
    for chunk in CHUNKS:
        slots = chunk // P
        g = imp_core[row0:row0 + chunk].reshape(P, slots)
        gi = np.transpose(g, (1, 0)).reshape(chunk)      # position c*P + p
        cols.append(gi.reshape(chunk // 16, 16).T)       # [16, chunk//16]
        row0 += chunk
    w = np.concatenate(cols, axis=1)                     # [16, SHARD//16]
    return np.ascontiguousarray(np.tile(w, (8, 1)).astype(np.int16))


def make_in_maps(image_outputs, audio_outputs, I_imp_ind, A_imp_ind):
    img = np.ascontiguousarray(image_outputs, dtype=np.float32)
    aud = np.ascontiguousarray(audio_outputs, dtype=np.float32)
    I_imp = np.asarray(I_imp_ind).astype(np.int64)
    A_imp = np.asarray(A_imp_ind).astype(np.int64)
    in_maps = []
    for c in range(NCORES):
        base = c * SHARD
        in_maps.append({
            "img_full": img,
            "aud_full": aud,
            "img_loc": np.ascontiguousarray(img[base:base + SHARD]),
            "aud_loc": np.ascontiguousarray(aud[base:base + SHARD]),
            "iidx": _prep_idx(I_imp[base:base + SHARD]),
            "aidx": _prep_idx(A_imp[base:base + SHARD]),
        })
    return in_maps


def kernel(image_outputs, audio_outputs, I_imp_ind, A_imp_ind):
    from concourse import bass_utils

    nc = _get_nc()
    in_maps = make_in_maps(image_outputs, audio_outputs, I_imp_ind, A_imp_ind)
    res = bass_utils.run_bass_kernel_spmd(nc, in_maps, list(range(NCORES))).results
    total = sum(float(r["partial"].sum(dtype=np.float64)) for r in res)
    return np.float32(total / N)
